# revision 20
# baseline (speedup 1.0000x reference)
"""Sparse-attention Trainium2 kernel (nn_AttentionLayer, B=16 S=2048 D=128).

reference semantics:
    A = Q @ T^T                     # [B,S,S]
    A = where(A > 0.3, A, 0)
    A += where(strictly_upper, -2^32, 0)
    y = softmax(A / sqrt(D)) @ V

Sharding: data-parallel over batch, 2 batches per core on 8 NeuronCores.

v4: permuted contiguous loads.
  All inputs are loaded with per-partition-contiguous DMA patterns so
  triggers are cheap and transfers fast:
    Q[b]: slab s (512 rows), q = 512s + 4p + jq   -> [128p, 4jq, 128d]
    T[b], V[b]: half h (1024 rows), k = 512K + 4p + jj
                                               -> [128p, (K jj)=8, 128d]
  A "k-chunk" c (=4K+jj) holds keys {512K + 4p + jj : p}. Scores are
  computed transposed S^T[k-chunk, q] with q columns enumerated
  p-major so queries appear in NATURAL order (q = 512*qb + col).
  Chunks with K == qb straddle the diagonal; their causal mask is a
  4-row staircase R01[jj][pk, col] = col >= 4*pk + jj, applied as a
  bf16 multiply after exp+max (masked garbage scores are zeroed then).

  num = max(exp(S^T*scale),1): ScalarE exp [128,1024] per group
  (fp32 PSUM -> bf16 SBUF), VectorE tensor_scalar_max (4x mode).
  PV + denominator per (chunk, q-subtile): lhsT = num chunk, rhs =
  [V | ones] [128k,129], PSUM-accumulated. out = PV/den via VectorE
  PSUM->SBUF copy + GpSimd normalize_recip. Output store is natural.

  Queue discipline (DMA queues are FIFO; any dep on a DMA waits for
  ALL earlier DMAs on that queue):
    scalar queue: batch-0 loads only (8 cheap triggers, done ~11us,
      ScalarE then runs exp undisturbed).
    sync queue: xbar transposes first (nothing bulky ahead of them),
      then batch-1 whole-tensor loads, then output stores.
"""

from collections import deque
from contextlib import ExitStack

import numpy as np

import concourse.bass as bass
import concourse.mybir as mybir
import concourse.tile as tile
from concourse import bacc

B, S, D = 16, 2048, 128
N_CORES = 8
B_LOC = B // N_CORES
QB = 512
N_QB = S // QB
SCALE = float(1.0 / np.sqrt(D))

F32 = mybir.dt.float32
BF16 = mybir.dt.bfloat16
Alu = mybir.AluOpType


def build_attention_core():
    nc = bacc.Bacc("TRN2", target_bir_lowering=False, debug=False,
                   num_devices=N_CORES)
    q_ext = nc.dram_tensor("Q", [B_LOC, S, D], F32, kind="ExternalInput").ap()
    t_ext = nc.dram_tensor("T", [B_LOC, S, D], F32, kind="ExternalInput").ap()
    v_ext = nc.dram_tensor("V", [B_LOC, S, D], F32, kind="ExternalInput").ap()
    o_ext = nc.dram_tensor("out", [B_LOC, S, D], F32, kind="ExternalOutput").ap()

    with tile.TileContext(nc) as tc, ExitStack() as ctx:
        const_pool = ctx.enter_context(tc.tile_pool(name="const", bufs=1))
        nat_pool = ctx.enter_context(tc.tile_pool(name="nat", bufs=1))
        stage_pool = ctx.enter_context(tc.tile_pool(name="stage", bufs=1))
        tpd_pool = ctx.enter_context(tc.tile_pool(name="tpd", bufs=1))
        vb_pool = ctx.enter_context(tc.tile_pool(name="vb", bufs=1))
        num_pool = ctx.enter_context(tc.tile_pool(name="num", bufs=6))
        fin_pool = ctx.enter_context(tc.tile_pool(name="fin", bufs=3))
        rec_pool = ctx.enter_context(tc.tile_pool(name="rec", bufs=4))
        qk_psum = ctx.enter_context(tc.tile_pool(name="qk_ps", bufs=2, space="PSUM"))
        ob_psum = ctx.enter_context(tc.tile_pool(name="ob_ps", bufs=4, space="PSUM"))

        # ---- constants (gpsimd) ----
        junk = const_pool.tile([128, 512], BF16, name="junk")
        nc.gpsimd.memset(junk[:], 0.25)
        # r01p[i][pk, 512*j + col] = 1 if col >= 4*pk + (2i+j) else 0 —
        # the diag keep-masks for a group's chunk pair (jj = 2i, 2i+1)
        r01p = []
        for i in range(2):
            m = const_pool.tile([128, 2, 512], BF16, name=f"r01p_{i}")
            nc.gpsimd.memset(m[:], 1.0)
            for j in range(2):
                nc.gpsimd.affine_select(
                    out=m[:, j, :], in_=m[:, j, :],
                    compare_op=Alu.is_ge, fill=0.0,
                    base=-(2 * i + j), channel_multiplier=-4,
                    pattern=[[1, 512]])
            r01p.append(m)

        # ---- staging tensors ----
        # Chunk-pair staging: stg[b][c2] holds [t half c2 | q half c2] in
        # 16 tile-slots (t k-chunks 8c2..8c2+8 at slots 0:8, q slabs
        # 2c2,2c2+1 at slots 8:16) so ONE xbar call transposes both.
        # batch-0: per-half nat tensors (own DMA each); batch-1: whole.
        tnat0 = [nat_pool.tile([128, 8, 128], F32, name=f"tn0h{h}")
                 for h in range(2)]
        qnat0 = [nat_pool.tile([128, 8, 128], F32, name=f"qn0h{h}")
                 for h in range(2)]
        vnat0 = [nat_pool.tile([128, 8, 128], F32, name=f"vn0h{h}")
                 for h in range(2)]
        nat1 = {w: nat_pool.tile([128, 16, 128], F32, name=f"n1{w}")
                for w in ("q", "t", "v")}

        stg = [[stage_pool.tile([128, 16, 128], BF16, name=f"stg{b}c{c}")
                for c in range(2)] for b in range(2)]
        tp = [[tpd_pool.tile([128, 16, 128], BF16, name=f"tp{b}c{c}")
               for c in range(2)] for b in range(2)]
        vaug = [[vb_pool.tile([128, 8, 129], BF16, name=f"va{b}h{h}")
                 for h in range(2)] for b in range(2)]
        for b in range(2):
            for h in range(2):
                nc.gpsimd.memset(vaug[b][h][:, :, D:D + 1], 1.0)

        # ---- load helpers (contiguous pattern: row = 512K + 4p + j) ----
        def load_half0(which, h):
            ext = {"q": q_ext, "t": t_ext, "v": v_ext}[which]
            dst = {"q": qnat0, "t": tnat0, "v": vnat0}[which][h]
            nc.scalar.dma_start(
                dst[:].rearrange("p (K j) d -> p K j d", K=2),
                ext[0, 1024 * h:1024 * (h + 1), :]
                .rearrange("(K p j) d -> p K j d", p=128, j=4))

        def load_b1(which):
            ext = {"q": q_ext, "t": t_ext, "v": v_ext}[which]
            nc.sync.dma_start(
                nat1[which][:].rearrange("p (K j) d -> p K j d", K=4),
                ext[1].rearrange("(K p j) d -> p K j d", p=128, j=4))

        def cast_t(b, c2):
            src = tnat0[c2][:] if b == 0 else nat1["t"][:, 8 * c2:8 * c2 + 8, :]
            nc.vector.tensor_copy(stg[b][c2][:, 0:8, :], src)

        def cast_q(b, c2):
            src = qnat0[c2][:] if b == 0 else nat1["q"][:, 8 * c2:8 * c2 + 8, :]
            nc.vector.tensor_copy(stg[b][c2][:, 8:16, :], src)

        def cast_v(b, h):
            src = vnat0[h][:] if b == 0 else nat1["v"][:, 8 * h:8 * h + 8, :]
            nc.vector.tensor_copy(vaug[b][h][:, :, 0:D], src)

        def xpose(b, c2):
            nc.sync.dma_start_transpose(
                tp[b][c2][:], stg[b][c2][:].rearrange("p t d -> p (t d)"))

        # ---- PE warm-up: ramp the p-state while DMA prep runs ----
        for w in range(14):
            wps = qk_psum.tile([128, 1024], F32, tag="qk", name=f"wps{w}")
            nc.tensor.matmul(wps[:, 0:512], lhsT=junk[:, 0:128], rhs=junk[:])

        # ---- batch-0 loads on the scalar queue, critical-first ----
        load_half0("t", 0)
        load_half0("q", 0)
        load_half0("v", 0)
        load_half0("t", 1)
        load_half0("q", 1)
        load_half0("v", 1)

        cast_t(0, 0)
        cast_q(0, 0)
        cast_v(0, 0)
        cast_t(0, 1)
        cast_q(0, 1)
        cast_v(0, 1)

        # sync queue: transposes first (FIFO kept clear), then b1 loads
        xpose(0, 0)
        xpose(0, 1)
        # hint the scheduler to keep b1 loads behind the b0 transposes on
        # the sync FIFO (and off batch-0's HBM bandwidth)
        with tc.tile_wait_until(0.014):
            load_b1("t")
            load_b1("q")
            load_b1("v")

        items = []
        for b in range(B_LOC):
            for qb in range(N_QB):
                for g in range((4 * qb + 4) // 2):
                    items.append((b, qb, g))

        def fillers(n):
            for _ in range(n):
                wps = qk_psum.tile([128, 1024], F32, tag="qk")
                nc.tensor.matmul(wps[:, 0:512], lhsT=junk[:, 0:128],
                                 rhs=junk[:])

        def prep_b1(step):
            if step == 0:
                cast_t(1, 0)
            elif step == 1:
                cast_q(1, 0)
            elif step == 2:
                xpose(1, 0)
            elif step == 3:
                cast_v(1, 0)
            elif step == 4:
                cast_t(1, 1)
            elif step == 5:
                cast_q(1, 1)
            elif step == 6:
                xpose(1, 1)
            elif step == 7:
                cast_v(1, 1)

        prep_at = {13: 0, 14: 1, 15: 2, 16: 3, 17: 4, 18: 5, 19: 6, 22: 7}

        state = {}

        def qk_group(b, qb, g):
            s_ps = qk_psum.tile([128, 1024], F32, tag="qk")
            num = num_pool.tile([128, 1024], BF16, tag="num")
            qs = 8 + 4 * (qb % 2)
            rhs = tp[b][qb // 2][:, qs:qs + 4, :].rearrange("d j p -> d p j")
            for j, c in enumerate((2 * g, 2 * g + 1)):
                nc.tensor.matmul(
                    s_ps[:, j * 512:j * 512 + 512],
                    lhsT=tp[b][c // 8][:, c % 8, :],
                    rhs=rhs,
                    start=True, stop=True,
                )
            nc.scalar.activation(num[:], s_ps[:],
                                 mybir.ActivationFunctionType.Exp,
                                 scale=SCALE)
            nc.vector.tensor_scalar_max(num[:], num[:], 1.0)
            i0 = 2 * g - 4 * qb  # chunk pair is diagonal iff i0 >= 0
            if i0 >= 0:
                nc.vector.tensor_tensor(
                    num[:], num[:],
                    r01p[i0 // 2][:].rearrange("p a c -> p (a c)"),
                    op=Alu.mult)
            st = state.setdefault((b, qb), {"ob": None, "num": {}})
            if st["ob"] is None:
                st["ob"] = [ob_psum.tile([128, 2, 256], F32, tag="ob",
                                         name=f"ob_{b}_{qb}_{h}")
                            for h in range(2)]
            st["num"][g] = num

        def pv_group(b, qb, g):
            st = state[(b, qb)]
            num = st["num"].pop(g)
            for j, c in enumerate((2 * g, 2 * g + 1)):
                s0 = j * 512
                for sub in range(4):
                    ob = st["ob"][sub // 2]
                    nc.tensor.matmul(
                        ob[:, sub % 2, 0:129],
                        lhsT=num[:, s0 + sub * 128:s0 + (sub + 1) * 128],
                        rhs=vaug[b][c // 8][:, c % 8, 0:129],
                        start=(c == 0 and sub % 2 == 0),
                        stop=(c == 4 * qb + 3),
                        skip_group_check=True,
                    )

        def finalize(b, qb):
            st = state.pop((b, qb))
            o_tile = fin_pool.tile([128, 4, 128], F32, tag="fin")
            for h in range(2):
                ob_sb = rec_pool.tile([128, 2, 129], F32, tag="rec")
                nc.vector.tensor_copy(ob_sb[:], st["ob"][h][:, :, 0:129])
                for s2 in range(2):
                    nc.gpsimd.normalize_recip(
                        o_tile[:, 2 * h + s2, :],
                        ob_sb[:, s2, 0:128],
                        ob_sb[:, s2, 128:129])
            nc.sync.dma_start(
                o_ext[b, qb * QB:(qb + 1) * QB, :]
                    .rearrange("(s p) d -> p s d", p=128),
                o_tile[:])

        pending = deque()

        def flush_one():
            b, qb, g = pending.popleft()
            pv_group(b, qb, g)
            if g == (4 * qb + 4) // 2 - 1:
                finalize(b, qb)

        n_items = len(items)
        for idx, it in enumerate(items):
            qk_group(*it)
            if idx in prep_at:
                prep_b1(prep_at[idx])
            pending.append(it)
            depth = 2 if idx < n_items - 4 else 1
            while len(pending) > depth:
                flush_one()
        while pending:
            flush_one()

    nc.compile()
    return nc


_NC_CACHE = None


def _get_nc():
    global _NC_CACHE
    if _NC_CACHE is None:
        _NC_CACHE = build_attention_core()
    return _NC_CACHE


def kernel(Q: np.ndarray, T: np.ndarray, V: np.ndarray) -> np.ndarray:
    """Full-input entry point: shard over batch, run 8-core SPMD, gather."""
    from concourse.bass_utils import run_bass_kernel_spmd

    Q = np.ascontiguousarray(np.asarray(Q, dtype=np.float32))
    T = np.ascontiguousarray(np.asarray(T, dtype=np.float32))
    V = np.ascontiguousarray(np.asarray(V, dtype=np.float32))
    assert Q.shape == (B, S, D), Q.shape

    nc = _get_nc()
    in_maps = [
        {
            "Q": Q[i * B_LOC:(i + 1) * B_LOC],
            "T": T[i * B_LOC:(i + 1) * B_LOC],
            "V": V[i * B_LOC:(i + 1) * B_LOC],
        }
        for i in range(N_CORES)
    ]
    res = run_bass_kernel_spmd(nc, in_maps, core_ids=list(range(N_CORES)))
    return np.concatenate([res.results[i]["out"] for i in range(N_CORES)], axis=0)


# revision 24
# speedup vs baseline: 1.6482x; 1.6482x over previous
"""Sparse-attention Trainium2 kernel (nn_AttentionLayer, B=16 S=2048 D=128).

reference semantics:
    A = Q @ T^T                     # [B,S,S]
    A = where(A > 0.3, A, 0)
    A += where(strictly_upper, -2^32, 0)
    y = softmax(A / sqrt(D)) @ V

Sharding: data-parallel over batch, 2 batches per core on 8 NeuronCores.

v4: permuted contiguous loads.
  All inputs are loaded with per-partition-contiguous DMA patterns so
  triggers are cheap and transfers fast:
    Q[b]: slab s (512 rows), q = 512s + 4p + jq   -> [128p, 4jq, 128d]
    T[b], V[b]: half h (1024 rows), k = 512K + 4p + jj
                                               -> [128p, (K jj)=8, 128d]
  A "k-chunk" c (=4K+jj) holds keys {512K + 4p + jj : p}. Scores are
  computed transposed S^T[k-chunk, q] with q columns enumerated
  p-major so queries appear in NATURAL order (q = 512*qb + col).
  Chunks with K == qb straddle the diagonal; their causal mask is a
  4-row staircase R01[jj][pk, col] = col >= 4*pk + jj, applied as a
  bf16 multiply after exp+max (masked garbage scores are zeroed then).

  num = max(exp(S^T*scale),1): ScalarE exp [128,1024] per group
  (fp32 PSUM -> bf16 SBUF), VectorE tensor_scalar_max (4x mode).
  PV + denominator per (chunk, q-subtile): lhsT = num chunk, rhs =
  [V | ones] [128k,129], PSUM-accumulated. out = PV/den via VectorE
  PSUM->SBUF copy + GpSimd normalize_recip. Output store is natural.

  Queue discipline (DMA queues are FIFO; any dep on a DMA waits for
  ALL earlier DMAs on that queue):
    scalar queue: batch-0 loads only (8 cheap triggers, done ~11us,
      ScalarE then runs exp undisturbed).
    sync queue: xbar transposes first (nothing bulky ahead of them),
      then batch-1 whole-tensor loads, then output stores.
"""

from collections import deque
from contextlib import ExitStack

import numpy as np

import concourse.bass as bass
import concourse.mybir as mybir
import concourse.tile as tile
from concourse import bacc

B, S, D = 16, 2048, 128
N_CORES = 8
B_LOC = B // N_CORES
QB = 512
N_QB = S // QB
SCALE = float(1.0 / np.sqrt(D))

F32 = mybir.dt.float32
BF16 = mybir.dt.bfloat16
Alu = mybir.AluOpType


def build_attention_core():
    nc = bacc.Bacc("TRN2", target_bir_lowering=False, debug=False,
                   num_devices=N_CORES)
    q_ext = nc.dram_tensor("Q", [B_LOC, S, D], F32, kind="ExternalInput").ap()
    t_ext = nc.dram_tensor("T", [B_LOC, S, D], F32, kind="ExternalInput").ap()
    v_ext = nc.dram_tensor("V", [B_LOC, S, D], F32, kind="ExternalInput").ap()
    o_ext = nc.dram_tensor("out", [B_LOC, S, D], F32, kind="ExternalOutput").ap()

    with tile.TileContext(nc) as tc, ExitStack() as ctx:
        const_pool = ctx.enter_context(tc.tile_pool(name="const", bufs=1))
        nat_pool = ctx.enter_context(tc.tile_pool(name="nat", bufs=1))
        stage_pool = ctx.enter_context(tc.tile_pool(name="stage", bufs=1))
        tpd_pool = ctx.enter_context(tc.tile_pool(name="tpd", bufs=1))
        vb_pool = ctx.enter_context(tc.tile_pool(name="vb", bufs=1))
        num_pool = ctx.enter_context(tc.tile_pool(name="num", bufs=6))
        fin_pool = ctx.enter_context(tc.tile_pool(name="fin", bufs=3))
        rec_pool = ctx.enter_context(tc.tile_pool(name="rec", bufs=4))
        qk_psum = ctx.enter_context(tc.tile_pool(name="qk_ps", bufs=2, space="PSUM"))
        ob_psum = ctx.enter_context(tc.tile_pool(name="ob_ps", bufs=4, space="PSUM"))

        # ---- constants (gpsimd) ----
        junk = const_pool.tile([128, 512], BF16, name="junk")
        nc.gpsimd.memset(junk[:], 0.25)
        # Score columns are enumerated (j outer, p inner): col = 128j + p
        # <-> query 512qb + 4p + j. Diagonal keep-mask for chunk jj:
        # keep iff 4p + j >= 4pk + jj. r01p[i] covers the group's chunk
        # pair (jj = 2i, 2i+1), one 512-wide half per chunk.
        r01p = []
        for i in range(2):
            m = const_pool.tile([128, 2, 4, 128], BF16, name=f"r01p_{i}")
            nc.gpsimd.memset(m[:], 1.0)
            for h in range(2):
                nc.gpsimd.affine_select(
                    out=m[:, h, :, :], in_=m[:, h, :, :],
                    compare_op=Alu.is_ge, fill=0.0,
                    base=-(2 * i + h), channel_multiplier=-4,
                    pattern=[[1, 4], [4, 128]])
            r01p.append(m)

        # ---- staging tensors ----
        # Chunk-pair staging: stg[b][c2] holds [t half c2 | q half c2] in
        # 16 tile-slots (t k-chunks 8c2..8c2+8 at slots 0:8, q slabs
        # 2c2,2c2+1 at slots 8:16) so ONE xbar call transposes both.
        # batch-0: per-half nat tensors (own DMA each); batch-1: whole.
        tnat0 = [nat_pool.tile([128, 8, 128], F32, name=f"tn0h{h}")
                 for h in range(2)]
        qnat0 = [nat_pool.tile([128, 8, 128], F32, name=f"qn0h{h}")
                 for h in range(2)]
        vnat0 = [nat_pool.tile([128, 8, 128], F32, name=f"vn0h{h}")
                 for h in range(2)]
        nat1 = {w: nat_pool.tile([128, 16, 128], F32, name=f"n1{w}")
                for w in ("q", "t", "v")}

        stg = [[stage_pool.tile([128, 16, 128], BF16, name=f"stg{b}c{c}")
                for c in range(2)] for b in range(2)]
        tp = [[tpd_pool.tile([128, 16, 128], BF16, name=f"tp{b}c{c}")
               for c in range(2)] for b in range(2)]
        vaug = [[vb_pool.tile([128, 8, 129], BF16, name=f"va{b}h{h}")
                 for h in range(2)] for b in range(2)]
        for b in range(2):
            for h in range(2):
                nc.gpsimd.memset(vaug[b][h][:, :, D:D + 1], 1.0)

        # ---- load helpers (contiguous pattern: row = 512K + 4p + j) ----
        def load_half0(which, h):
            ext = {"q": q_ext, "t": t_ext, "v": v_ext}[which]
            dst = {"q": qnat0, "t": tnat0, "v": vnat0}[which][h]
            nc.scalar.dma_start(
                dst[:].rearrange("p (K j) d -> p K j d", K=2),
                ext[0, 1024 * h:1024 * (h + 1), :]
                .rearrange("(K p j) d -> p K j d", p=128, j=4))

        def load_b1(which):
            ext = {"q": q_ext, "t": t_ext, "v": v_ext}[which]
            nc.sync.dma_start(
                nat1[which][:].rearrange("p (K j) d -> p K j d", K=4),
                ext[1].rearrange("(K p j) d -> p K j d", p=128, j=4))

        def cast_t(b, c2):
            src = tnat0[c2][:] if b == 0 else nat1["t"][:, 8 * c2:8 * c2 + 8, :]
            nc.vector.tensor_copy(stg[b][c2][:, 0:8, :], src)

        def cast_q(b, c2):
            src = qnat0[c2][:] if b == 0 else nat1["q"][:, 8 * c2:8 * c2 + 8, :]
            nc.vector.tensor_copy(stg[b][c2][:, 8:16, :], src)

        def cast_v(b, h):
            src = vnat0[h][:] if b == 0 else nat1["v"][:, 8 * h:8 * h + 8, :]
            nc.vector.tensor_copy(vaug[b][h][:, :, 0:D], src)

        def xpose(b, c2):
            nc.sync.dma_start_transpose(
                tp[b][c2][:], stg[b][c2][:].rearrange("p t d -> p (t d)"))

        # ---- PE warm-up: ramp the p-state while DMA prep runs ----
        for w in range(14):
            wps = qk_psum.tile([128, 1024], F32, tag="qk", name=f"wps{w}")
            nc.tensor.matmul(wps[:, 0:512], lhsT=junk[:, 0:128], rhs=junk[:])

        # ---- batch-0 loads on the scalar queue, critical-first ----
        load_half0("t", 0)
        load_half0("q", 0)
        load_half0("v", 0)
        load_half0("t", 1)
        load_half0("q", 1)
        load_half0("v", 1)

        cast_t(0, 0)
        cast_q(0, 0)
        cast_v(0, 0)
        cast_t(0, 1)
        cast_q(0, 1)
        cast_v(0, 1)

        # sync queue: transposes first (FIFO kept clear), then b1 loads
        xpose(0, 0)
        xpose(0, 1)
        # hint the scheduler to keep b1 loads behind the b0 transposes on
        # the sync FIFO (and off batch-0's HBM bandwidth)
        with tc.tile_wait_until(0.014):
            load_b1("t")
            load_b1("q")
            load_b1("v")

        items = []
        for b in range(B_LOC):
            for qb in range(N_QB):
                for g in range((4 * qb + 4) // 2):
                    items.append((b, qb, g))

        def fillers(n):
            for _ in range(n):
                wps = qk_psum.tile([128, 1024], F32, tag="qk")
                nc.tensor.matmul(wps[:, 0:512], lhsT=junk[:, 0:128],
                                 rhs=junk[:])

        def prep_b1(step):
            if step == 0:
                cast_t(1, 0)
            elif step == 1:
                cast_q(1, 0)
            elif step == 2:
                xpose(1, 0)
            elif step == 3:
                cast_v(1, 0)
            elif step == 4:
                cast_t(1, 1)
            elif step == 5:
                cast_q(1, 1)
            elif step == 6:
                xpose(1, 1)
            elif step == 7:
                cast_v(1, 1)

        prep_at = {13: 0, 14: 1, 15: 2, 16: 3, 17: 4, 18: 5, 19: 6, 22: 7}

        state = {}

        def qk_group(b, qb, g):
            s_ps = qk_psum.tile([128, 1024], F32, tag="qk")
            num = num_pool.tile([128, 1024], BF16, tag="num")
            qs = 8 + 4 * (qb % 2)
            rhs = tp[b][qb // 2][:, qs:qs + 4, :].rearrange("d j p -> d (j p)")
            for j, c in enumerate((2 * g, 2 * g + 1)):
                nc.tensor.matmul(
                    s_ps[:, j * 512:j * 512 + 512],
                    lhsT=tp[b][c // 8][:, c % 8, :],
                    rhs=rhs,
                    start=True, stop=True,
                )
            nc.scalar.activation(num[:], s_ps[:],
                                 mybir.ActivationFunctionType.Exp,
                                 scale=SCALE)
            nc.vector.tensor_scalar_max(num[:], num[:], 1.0)
            i0 = 2 * g - 4 * qb  # chunk pair is diagonal iff i0 >= 0
            if i0 >= 0:
                nc.vector.tensor_tensor(
                    num[:], num[:],
                    r01p[i0 // 2][:].rearrange("p a j q -> p (a j q)"),
                    op=Alu.mult)
            st = state.setdefault((b, qb), {"ob": None, "num": {}})
            if st["ob"] is None:
                st["ob"] = [ob_psum.tile([128, 2, 256], F32, tag="ob",
                                         name=f"ob_{b}_{qb}_{h}")
                            for h in range(2)]
            st["num"][g] = num

        def pv_group(b, qb, g):
            st = state[(b, qb)]
            num = st["num"].pop(g)
            for j, c in enumerate((2 * g, 2 * g + 1)):
                s0 = j * 512
                for sub in range(4):
                    ob = st["ob"][sub // 2]
                    nc.tensor.matmul(
                        ob[:, sub % 2, 0:129],
                        lhsT=num[:, s0 + sub * 128:s0 + (sub + 1) * 128],
                        rhs=vaug[b][c // 8][:, c % 8, 0:129],
                        start=(c == 0 and sub % 2 == 0),
                        stop=(c == 4 * qb + 3),
                        skip_group_check=True,
                    )

        def finalize(b, qb):
            st = state.pop((b, qb))
            o_tile = fin_pool.tile([128, 4, 128], F32, tag="fin")
            for h in range(2):
                ob_sb = rec_pool.tile([128, 2, 129], F32, tag="rec")
                nc.vector.tensor_copy(ob_sb[:], st["ob"][h][:, :, 0:129])
                for s2 in range(2):
                    nc.gpsimd.normalize_recip(
                        o_tile[:, 2 * h + s2, :],
                        ob_sb[:, s2, 0:128],
                        ob_sb[:, s2, 128:129])
            # ob row r of subtile u is query 512qb + 4r + u, so partition
            # r's 4 rows are DRAM-contiguous (2KB store packets)
            nc.sync.dma_start(
                o_ext[b, qb * QB:(qb + 1) * QB, :]
                    .rearrange("(p j) d -> p j d", p=128),
                o_tile[:])

        pending = deque()

        def flush_one():
            b, qb, g = pending.popleft()
            pv_group(b, qb, g)
            if g == (4 * qb + 4) // 2 - 1:
                finalize(b, qb)

        n_items = len(items)
        for idx, it in enumerate(items):
            qk_group(*it)
            if idx in prep_at:
                prep_b1(prep_at[idx])
            pending.append(it)
            depth = 2 if idx < n_items - 4 else 1
            while len(pending) > depth:
                flush_one()
        while pending:
            flush_one()

    nc.compile()
    return nc


_NC_CACHE = None


def _get_nc():
    global _NC_CACHE
    if _NC_CACHE is None:
        _NC_CACHE = build_attention_core()
    return _NC_CACHE


def kernel(Q: np.ndarray, T: np.ndarray, V: np.ndarray) -> np.ndarray:
    """Full-input entry point: shard over batch, run 8-core SPMD, gather."""
    from concourse.bass_utils import run_bass_kernel_spmd

    Q = np.ascontiguousarray(np.asarray(Q, dtype=np.float32))
    T = np.ascontiguousarray(np.asarray(T, dtype=np.float32))
    V = np.ascontiguousarray(np.asarray(V, dtype=np.float32))
    assert Q.shape == (B, S, D), Q.shape

    nc = _get_nc()
    in_maps = [
        {
            "Q": Q[i * B_LOC:(i + 1) * B_LOC],
            "T": T[i * B_LOC:(i + 1) * B_LOC],
            "V": V[i * B_LOC:(i + 1) * B_LOC],
        }
        for i in range(N_CORES)
    ]
    res = run_bass_kernel_spmd(nc, in_maps, core_ids=list(range(N_CORES)))
    return np.concatenate([res.results[i]["out"] for i in range(N_CORES)], axis=0)


# revision 29
# speedup vs baseline: 1.6996x; 1.0312x over previous
"""Sparse-attention Trainium2 kernel (nn_AttentionLayer, B=16 S=2048 D=128).

reference semantics:
    A = Q @ T^T                     # [B,S,S]
    A = where(A > 0.3, A, 0)
    A += where(strictly_upper, -2^32, 0)
    y = softmax(A / sqrt(D)) @ V

Sharding: data-parallel over batch, 2 batches per core on 8 NeuronCores.

Per-core algorithm (per batch), final:
  - Q, T cast to bf16 on VectorE into per-chunk staging tiles (separate
    tensors so the DMA xbar transposes' coarse-grained read deps don't
    serialize), transposed SBUF->SBUF by 3 large xbar calls per batch.
    Load DMAs are issued from sync/scalar/vector queues in parallel.
  - Scores computed transposed, S^T[k,q], 2 ktiles per [128,1024] PSUM
    tile. Straddling-diagonal k-tiles skip dead query columns and are
    left-packed in their PSUM bank so exp spans merge.
  - num = max(exp(S^T*scale),1): ScalarE exp (fp32 PSUM -> bf16 SBUF),
    VectorE tensor_scalar_max (4x mode). The causal mask of each
    diagonal 128x128 block is a VectorE multiply with a 0/1 triangle
    (GpSimd runs ONLY normalize_recip: mixing op families on GpSimd
    costs a ~6.5us ucode library swap per switch).
  - PV + denominator fused per (ktile, q-subtile): lhsT = num chunk,
    rhs = [V | ones] [128k,129], PSUM-accumulated; obanks packed two
    per PSUM bank (only the bank's first matmul sets start: start=True
    arms a bank-wide lazy zero).
  - out = PV/den via a VectorE PSUM->SBUF copy + GpSimd normalize_recip.
  - PE warm-up matmuls ramp the p-state during the DMA prep; PV for
    group g is emitted two QK groups later (software pipelining).
"""

from collections import deque
from contextlib import ExitStack

import numpy as np

import concourse.bass as bass
import concourse.mybir as mybir
import concourse.tile as tile
from concourse import bacc

B, S, D = 16, 2048, 128
N_CORES = 8
B_LOC = B // N_CORES
QB = 512
KT = 128
N_QB = S // QB
N_ST = S // 128
SCALE = float(1.0 / np.sqrt(D))

F32 = mybir.dt.float32
BF16 = mybir.dt.bfloat16
Alu = mybir.AluOpType


def build_attention_core():
    nc = bacc.Bacc("TRN2", target_bir_lowering=False, debug=False,
                   num_devices=N_CORES)
    q_ext = nc.dram_tensor("Q", [B_LOC, S, D], F32, kind="ExternalInput").ap()
    t_ext = nc.dram_tensor("T", [B_LOC, S, D], F32, kind="ExternalInput").ap()
    v_ext = nc.dram_tensor("V", [B_LOC, S, D], F32, kind="ExternalInput").ap()
    o_ext = nc.dram_tensor("out", [B_LOC, S, D], F32, kind="ExternalOutput").ap()

    with tile.TileContext(nc) as tc, ExitStack() as ctx:
        const_pool = ctx.enter_context(tc.tile_pool(name="const", bufs=1))
        nat_pool = ctx.enter_context(tc.tile_pool(name="nat", bufs=1))
        stage_pool = ctx.enter_context(tc.tile_pool(name="stage", bufs=1))
        tpd_pool = ctx.enter_context(tc.tile_pool(name="tpd", bufs=1))
        vb_pool = ctx.enter_context(tc.tile_pool(name="vb", bufs=1))
        num_pool = ctx.enter_context(tc.tile_pool(name="num", bufs=6))
        fin_pool = ctx.enter_context(tc.tile_pool(name="fin", bufs=3))
        rec_pool = ctx.enter_context(tc.tile_pool(name="rec", bufs=4))
        qk_psum = ctx.enter_context(tc.tile_pool(name="qk_ps", bufs=2, space="PSUM"))
        ob_psum = ctx.enter_context(tc.tile_pool(name="ob_ps", bufs=4, space="PSUM"))

        # ---- constants (gpsimd) ----
        junk = const_pool.tile([128, 512], BF16, name="junk")
        nc.gpsimd.memset(junk[:], 0.25)
        # tri01[p, n] = 0 if p > n else 1 (first 128 cols form the in-tile
        # causal keep-mask; cols >= 128 are all ones)
        # (v_aug ones columns are set at const time, below)
        tri01 = const_pool.tile([128, 128], BF16, name="tri01")
        nc.gpsimd.memset(tri01[:], 1.0)
        nc.gpsimd.affine_select(
            out=tri01[:], in_=tri01[:],
            compare_op=Alu.is_ge, fill=0.0,
            base=0, channel_multiplier=-1, pattern=[[1, 128]])

        # ---- PE warm-up: ramp the p-state while DMA prep runs ----
        for w in range(10):
            wps = qk_psum.tile([128, 1024], F32, tag="qk", name=f"wps{w}")
            nc.tensor.matmul(wps[:, 0:512], lhsT=junk[:, 0:128], rhs=junk[:])

        # ---- staging: per batch, 3 chunks (separate tensors so coarse
        # read deps never serialize): A=[q0:4|t0:4] B=[q4:8|t4:8]
        # C=[q8:16|t8:16]. QK rhs spans stay inside one chunk.
        CH = ((0, 4), (4, 4), (8, 8))  # (tile_lo, n) per chunk

        nats, stages, qt_tps, v_augs = [], [], [], []
        for b in range(B_LOC):
            natb = {w: [nat_pool.tile([128, n, D], F32,
                                      name=f"{w}nat{b}p{ci}")
                        for ci, (lo, n) in enumerate(CH)]
                    for w in ("q", "t", "v")}
            stgb = [stage_pool.tile([128, 2 * n, 128], BF16,
                                    name=f"stg{b}c{ci}")
                    for ci, (lo, n) in enumerate(CH)]
            tpb = [tpd_pool.tile([128, 2 * n, 128], BF16,
                                 name=f"tp{b}c{ci}")
                   for ci, (lo, n) in enumerate(CH)]
            vab = [vb_pool.tile([128, n, 129], BF16, name=f"vaug{b}p{ci}")
                   for ci, (lo, n) in enumerate(CH)]
            nats.append(natb); stages.append(stgb)
            qt_tps.append(tpb); v_augs.append(vab)

        def chunk_of(t):
            return 0 if t < 4 else (1 if t < 8 else 2)

        def q_tp(b, t):
            """(tensor, local slot) of transposed q tile t."""
            ci = chunk_of(t)
            return qt_tps[b][ci], t - CH[ci][0]

        def t_tp(b, c):
            ci = chunk_of(c)
            return qt_tps[b][ci], CH[ci][1] + c - CH[ci][0]

        def v_t(b, c):
            ci = chunk_of(c)
            return v_augs[b][ci], c - CH[ci][0]

        ext_of = {"q": q_ext, "t": t_ext, "v": v_ext}

        def load(b, which, ci, eng):
            lo, n = CH[ci]
            eng.dma_start(
                nats[b][which][ci][:],
                ext_of[which][b, 128 * lo:128 * (lo + n), :]
                .rearrange("(t p) d -> p t d", p=128))

        def cast_chunk(b, ci, which):
            """Cast q (slots 0:n) or t (slots n:2n) into staging chunk ci."""
            lo, n = CH[ci]
            off = 0 if which == "q" else n
            nc.vector.tensor_copy(stages[b][ci][:, off:off + n, :],
                                  nats[b][which][ci][:])

        for b in range(B_LOC):
            for ci in range(3):
                nc.gpsimd.memset(v_augs[b][ci][:, :, D:D + 1], 1.0)

        def cast_v(b, ci):
            nc.vector.tensor_copy(v_augs[b][ci][:, :, 0:D],
                                  nats[b]["v"][ci][:])

        def transpose(b, ci):
            nc.sync.dma_start_transpose(
                qt_tps[b][ci][:],
                stages[b][ci][:].rearrange("p t d -> p (t d)"))

        # ---- batch-0 head: q pieces on sync (kept clear for xbars),
        # t/v pieces on scalar (free until the first exp ~14us) ----
        load(0, "q", 0, nc.sync)
        load(0, "q", 1, nc.sync)
        load(0, "q", 2, nc.sync)
        load(0, "t", 0, nc.scalar)
        load(0, "v", 0, nc.scalar)
        load(0, "t", 1, nc.scalar)
        load(0, "v", 1, nc.scalar)
        load(0, "t", 2, nc.scalar)
        load(0, "v", 2, nc.scalar)

        cast_chunk(0, 0, "q")
        cast_chunk(0, 0, "t")
        cast_v(0, 0)
        cast_chunk(0, 1, "q")
        cast_chunk(0, 1, "t")
        cast_v(0, 1)
        cast_chunk(0, 2, "q")
        cast_chunk(0, 2, "t")

        transpose(0, 0)
        transpose(0, 1)
        transpose(0, 2)
        # batch-1 loads: hinted late so their transfers trail the xbars
        # on the sync FIFO (any dep on a DMA waits for all earlier DMAs
        # on its queue)
        with tc.tile_wait_until(0.02):
            load(1, "q", 0, nc.sync)
            load(1, "t", 0, nc.sync)
            load(1, "v", 0, nc.sync)
            load(1, "q", 1, nc.sync)
            load(1, "t", 1, nc.sync)
            load(1, "v", 1, nc.sync)
            load(1, "q", 2, nc.sync)
            load(1, "t", 2, nc.sync)
            load(1, "v", 2, nc.sync)

        items = []
        for b in range(B_LOC):
            for qb in range(N_QB):
                for g in range((4 * qb + 4) // 2):
                    items.append((b, qb, g))

        def prep_b1(step):
            if step == 0:
                cast_v(0, 2)
            elif step == 1:
                cast_chunk(1, 0, "q")
                cast_chunk(1, 0, "t")
            elif step == 2:
                transpose(1, 0)
                cast_v(1, 0)
            elif step == 3:
                cast_chunk(1, 1, "q")
                cast_chunk(1, 1, "t")
            elif step == 4:
                transpose(1, 1)
                cast_v(1, 1)
            elif step == 5:
                cast_chunk(1, 2, "q")
                cast_chunk(1, 2, "t")
            elif step == 6:
                transpose(1, 2)
            elif step == 7:
                cast_v(1, 2)

        prep_at = {3: 0, 14: 1, 15: 2, 16: 3, 17: 4, 18: 5, 19: 6, 21: 7}

        state = {}

        def qk_group(b, qb, g):
            q0 = qb * QB
            s_ps = qk_psum.tile([128, 1024], F32, tag="qk")
            num = num_pool.tile([128, 1024], BF16, tag="num")
            act_spans = []      # merged contiguous spans (left-packed)
            mask_blocks = []    # span starts of diagonal blocks
            last_g = (g == (4 * qb + 4) // 2 - 1)
            for j, c in enumerate((2 * g, 2 * g + 1)):
                i = c - 4 * qb
                lo = 128 * i if i > 0 else 0
                w = QB - lo
                ql = q0 + lo
                t0_ = ql // 128
                nt = (QB - lo) // 128
                # the final (i2,i3) pair packs into one bank: j1 at col 256
                s0 = 256 if (last_g and j == 1) else j * 512
                q_tens, q_lo = q_tp(b, t0_)
                rhs = q_tens[:, q_lo:q_lo + nt, :] \
                    .rearrange("p t q -> p (t q)")
                t_tens, t_lo = t_tp(b, c)
                nc.tensor.matmul(
                    s_ps[:, s0:s0 + w],
                    lhsT=t_tens[:, t_lo, :],
                    rhs=rhs,
                    start=not (last_g and j == 1), stop=True,
                    skip_group_check=(last_g and j == 1),
                )
                if act_spans and act_spans[-1][1] == s0:
                    act_spans[-1] = (act_spans[-1][0], s0 + w)
                else:
                    act_spans.append((s0, s0 + w))
                if i >= 0:
                    mask_blocks.append(s0)
            for lo_, hi_ in act_spans:
                nc.scalar.activation(num[:, lo_:hi_], s_ps[:, lo_:hi_],
                                     mybir.ActivationFunctionType.Exp,
                                     scale=SCALE)
                nc.vector.tensor_scalar_max(num[:, lo_:hi_],
                                            num[:, lo_:hi_], 1.0)
            for ds in mask_blocks:
                nc.vector.tensor_tensor(num[:, ds:ds + 128],
                                        num[:, ds:ds + 128], tri01[:],
                                        op=Alu.mult)
            st = state.setdefault((b, qb), {"ob": None, "num": {}})
            if st["ob"] is None:
                st["ob"] = [ob_psum.tile([128, 2, 256], F32, tag="ob",
                                         name=f"ob_{b}_{qb}_{h}")
                            for h in range(2)]
            st["num"][g] = num

        def pv_group(b, qb, g):
            st = state[(b, qb)]
            num = st["num"].pop(g)
            last_g = (g == (4 * qb + 4) // 2 - 1)
            for j, c in enumerate((2 * g, 2 * g + 1)):
                i = c - 4 * qb
                lo = 128 * i if i > 0 else 0
                s0 = 256 if (last_g and j == 1) else j * 512
                v_tens, v_lo = v_t(b, c)
                for sub in range(max(i, 0), 4):
                    ob = st["ob"][sub // 2]
                    nc.tensor.matmul(
                        ob[:, sub % 2, 0:129],
                        lhsT=num[:, s0 + sub * 128 - lo:
                                 s0 + (sub + 1) * 128 - lo],
                        rhs=v_tens[:, v_lo, 0:129],
                        start=(c == 0 and sub % 2 == 0),
                        stop=(c == 4 * qb + sub),
                        skip_group_check=True,
                    )

        def finalize(b, qb):
            st = state.pop((b, qb))
            o_tile = fin_pool.tile([128, 4, 128], F32, tag="fin")
            for h in range(2):
                ob_sb = rec_pool.tile([128, 2, 129], F32, tag="rec")
                nc.vector.tensor_copy(ob_sb[:], st["ob"][h][:, :, 0:129])
                for s2 in range(2):
                    nc.gpsimd.normalize_recip(
                        o_tile[:, 2 * h + s2, :],
                        ob_sb[:, s2, 0:128],
                        ob_sb[:, s2, 128:129])
            nc.sync.dma_start(
                o_ext[b, qb * QB:(qb + 1) * QB, :]
                    .rearrange("(s p) d -> p s d", p=128),
                o_tile[:])

        pending = deque()

        def flush_one():
            b, qb, g = pending.popleft()
            pv_group(b, qb, g)
            if g == (4 * qb + 4) // 2 - 1:
                finalize(b, qb)

        n_items = len(items)
        for idx, it in enumerate(items):
            qk_group(*it)
            if idx in prep_at:
                prep_b1(prep_at[idx])
            pending.append(it)
            # drain harder near the end so the tail is short
            depth = 2 if idx < n_items - 4 else 1
            while len(pending) > depth:
                flush_one()
        while pending:
            flush_one()

    nc.compile()
    return nc


_NC_CACHE = None


def _get_nc():
    global _NC_CACHE
    if _NC_CACHE is None:
        _NC_CACHE = build_attention_core()
    return _NC_CACHE


def kernel(Q: np.ndarray, T: np.ndarray, V: np.ndarray) -> np.ndarray:
    """Full-input entry point: shard over batch, run 8-core SPMD, gather."""
    from concourse.bass_utils import run_bass_kernel_spmd

    Q = np.ascontiguousarray(np.asarray(Q, dtype=np.float32))
    T = np.ascontiguousarray(np.asarray(T, dtype=np.float32))
    V = np.ascontiguousarray(np.asarray(V, dtype=np.float32))
    assert Q.shape == (B, S, D), Q.shape

    nc = _get_nc()
    in_maps = [
        {
            "Q": Q[i * B_LOC:(i + 1) * B_LOC],
            "T": T[i * B_LOC:(i + 1) * B_LOC],
            "V": V[i * B_LOC:(i + 1) * B_LOC],
        }
        for i in range(N_CORES)
    ]
    res = run_bass_kernel_spmd(nc, in_maps, core_ids=list(range(N_CORES)))
    return np.concatenate([res.results[i]["out"] for i in range(N_CORES)], axis=0)



# revision 32
# speedup vs baseline: 1.8724x; 1.1016x over previous
"""Sparse-attention Trainium2 kernel (nn_AttentionLayer, B=16 S=2048 D=128).

reference semantics:
    A = Q @ T^T                     # [B,S,S]
    A = where(A > 0.3, A, 0)
    A += where(strictly_upper, -2^32, 0)
    y = softmax(A / sqrt(D)) @ V

Sharding: data-parallel over batch, 2 batches per core on 8 NeuronCores.

Per-core algorithm (per batch), final:
  - Q, T cast to bf16 on VectorE into per-chunk staging tiles (separate
    tensors so the DMA xbar transposes' coarse-grained read deps don't
    serialize), transposed SBUF->SBUF by 3 large xbar calls per batch.
    Load DMAs are issued from sync/scalar/vector queues in parallel.
  - Scores computed transposed, S^T[k,q], 2 ktiles per [128,1024] PSUM
    tile. Straddling-diagonal k-tiles skip dead query columns and are
    left-packed in their PSUM bank so exp spans merge.
  - num = max(exp(S^T*scale),1): ScalarE exp (fp32 PSUM -> bf16 SBUF),
    VectorE tensor_scalar_max (4x mode). The causal mask of each
    diagonal 128x128 block is a VectorE multiply with a 0/1 triangle
    (GpSimd runs ONLY normalize_recip: mixing op families on GpSimd
    costs a ~6.5us ucode library swap per switch).
  - PV + denominator fused per (ktile, q-subtile): lhsT = num chunk,
    rhs = [V | ones] [128k,129], PSUM-accumulated; obanks packed two
    per PSUM bank (only the bank's first matmul sets start: start=True
    arms a bank-wide lazy zero).
  - out = PV/den via a VectorE PSUM->SBUF copy + GpSimd normalize_recip.
  - PE warm-up matmuls ramp the p-state during the DMA prep; PV for
    group g is emitted two QK groups later (software pipelining).
"""

from collections import deque
from contextlib import ExitStack

import numpy as np

import concourse.bass as bass
import concourse.mybir as mybir
import concourse.tile as tile
from concourse import bacc

B, S, D = 16, 2048, 128
N_CORES = 8
B_LOC = B // N_CORES
QB = 512
KT = 128
N_QB = S // QB
N_ST = S // 128
SCALE = float(1.0 / np.sqrt(D))

F32 = mybir.dt.float32
BF16 = mybir.dt.bfloat16
Alu = mybir.AluOpType


def build_attention_core():
    nc = bacc.Bacc("TRN2", target_bir_lowering=False, debug=False,
                   num_devices=N_CORES)
    q_ext = nc.dram_tensor("Q", [B_LOC, S, D], F32, kind="ExternalInput").ap()
    t_ext = nc.dram_tensor("T", [B_LOC, S, D], F32, kind="ExternalInput").ap()
    v_ext = nc.dram_tensor("V", [B_LOC, S, D], F32, kind="ExternalInput").ap()
    o_ext = nc.dram_tensor("out", [B_LOC, S, D], F32, kind="ExternalOutput").ap()

    with tile.TileContext(nc) as tc, ExitStack() as ctx:
        const_pool = ctx.enter_context(tc.tile_pool(name="const", bufs=1))
        nat_pool = ctx.enter_context(tc.tile_pool(name="nat", bufs=1))
        stage_pool = ctx.enter_context(tc.tile_pool(name="stage", bufs=1))
        tpd_pool = ctx.enter_context(tc.tile_pool(name="tpd", bufs=1))
        vb_pool = ctx.enter_context(tc.tile_pool(name="vb", bufs=1))
        num_pool = ctx.enter_context(tc.tile_pool(name="num", bufs=6))
        fin_pool = ctx.enter_context(tc.tile_pool(name="fin", bufs=3))
        rec_pool = ctx.enter_context(tc.tile_pool(name="rec", bufs=4))
        qk_psum = ctx.enter_context(tc.tile_pool(name="qk_ps", bufs=2, space="PSUM"))
        ob_psum = ctx.enter_context(tc.tile_pool(name="ob_ps", bufs=4, space="PSUM"))

        # ---- constants (gpsimd) ----
        junk = const_pool.tile([128, 512], BF16, name="junk")
        nc.gpsimd.memset(junk[:], 0.25)
        # tri01[p, n] = 0 if p > n else 1 (first 128 cols form the in-tile
        # causal keep-mask; cols >= 128 are all ones)
        # (v_aug ones columns are set at const time, below)
        tri01 = const_pool.tile([128, 128], BF16, name="tri01")
        nc.gpsimd.memset(tri01[:], 1.0)
        nc.gpsimd.affine_select(
            out=tri01[:], in_=tri01[:],
            compare_op=Alu.is_ge, fill=0.0,
            base=0, channel_multiplier=-1, pattern=[[1, 128]])

        # ---- PE warm-up: ramp the p-state while DMA prep runs ----
        for w in range(22):
            wps = qk_psum.tile([128, 1024], F32, tag="qk", name=f"wps{w}")
            nc.tensor.matmul(wps[:, 0:512], lhsT=junk[:, 0:128], rhs=junk[:])

        # ---- staging: per batch, 3 chunk tiles (separate tensors so the
        # xbar transpose read-dep doesn't wait on later casts) ----
        # chunk c0: [q0:4 | t0:4]; chunk cQ: q4:16; chunk cT: t4:16
        nats, stages, qt_tps, v_augs = [], [], [], []
        for b in range(B_LOC):
            q_nat = nat_pool.tile([128, N_ST, D], F32, name=f"qnat{b}")
            t_nat = nat_pool.tile([128, N_ST, D], F32, name=f"tnat{b}")
            v_nat = nat_pool.tile([128, N_ST, D], F32, name=f"vnat{b}")
            ca = stage_pool.tile([128, 16, 128], BF16, name=f"ca_{b}")
            cb = stage_pool.tile([128, 16, 128], BF16, name=f"cb_{b}")
            qt_tp = tpd_pool.tile([128, 2 * N_ST, 128], BF16, name=f"qttp{b}")
            v_aug = vb_pool.tile([128, N_ST, 129], BF16, name=f"vaug{b}")
            nats.append((q_nat, t_nat, v_nat))
            stages.append((ca, cb))
            qt_tps.append(qt_tp); v_augs.append(v_aug)

        # transposed slot layout (matches [cA | cB] = [q0:8|t0:8|q8:16|t8:16]):
        def q_slot(t):
            return t if t < 8 else 8 + t

        def t_slot(c):
            return 8 + c if c < 8 else 16 + c

        def load(b, which, h):
            q_nat, t_nat, v_nat = nats[b]
            nat = {"q": q_nat, "t": t_nat, "v": v_nat}[which]
            ext = {"q": q_ext, "t": t_ext, "v": v_ext}[which]
            # batch-0: q/t first halves on the sync ring (kept clear for the
            # xbar transposes), everything else on the scalar ring; batch-1
            # on sync with a scheduler hint so it can't hoist past the
            # transposes
            eng = nc.scalar if (b == 0 and (h == 1 or which == "v")) \
                else nc.sync
            sl = slice(0, 8) if h == 0 else slice(8, 16)
            ssl = slice(0, 1024) if h == 0 else slice(1024, 2048)
            eng.dma_start(
                nat[:, sl, :],
                ext[b, ssl, :].rearrange("(t p) d -> p t d", p=128))

        def cast_chunk(b, chunk, which):
            """Cast the q- or t-half of staging chunk (A: tiles 0:8, B: 8:16)."""
            q_nat, t_nat, v_nat = nats[b]
            nat = q_nat if which == "q" else t_nat
            stg = stages[b][chunk]
            off = 0 if which == "q" else 8
            nsl = slice(0, 8) if chunk == 0 else slice(8, 16)
            nc.vector.tensor_copy(stg[:, off:off + 8, :], nat[:, nsl, :])

        for b in range(B_LOC):
            nc.gpsimd.memset(v_augs[b][:, :, D:D + 1], 1.0)

        def cast_v(b, h):
            q_nat, t_nat, v_nat = nats[b]
            sl = slice(0, 8) if h == 0 else slice(8, 16)
            nc.vector.tensor_copy(v_augs[b][:, sl, 0:D], v_nat[:, sl, :])

        def transpose(b, chunk):
            src = stages[b][chunk]
            nc.sync.dma_start_transpose(
                qt_tps[b][:, 16 * chunk:16 * chunk + 16, :],
                src[:].rearrange("p t d -> p (t d)"))

        # ---- batch-0 head ----
        load(0, "v", 0)      # scalar queue, first so PV g0's V lands early
        load(0, "q", 0)      # sync
        load(0, "t", 0)      # sync
        load(0, "q", 1)      # scalar queue
        load(0, "t", 1)      # scalar queue
        load(0, "v", 1)      # scalar queue
        cast_chunk(0, 0, "q")
        cast_chunk(0, 0, "t")
        transpose(0, 0)
        cast_chunk(0, 1, "q")
        cast_chunk(0, 1, "t")
        transpose(0, 1)
        cast_v(0, 0)
        # batch-1 loads: scheduler-hinted late so their TRANSFERS also
        # queue behind both xbar transposes on the sync FIFO
        with tc.tile_wait_until(0.013):
            load(1, "q", 0)
            load(1, "t", 0)
            load(1, "v", 0)
            load(1, "q", 1)
            load(1, "t", 1)
            load(1, "v", 1)

        items = []
        for b in range(B_LOC):
            for qb in range(N_QB):
                for g in range((4 * qb + 4) // 2):
                    items.append((b, qb, g))

        prep_at = {
            2: lambda: cast_v(0, 1),
            16: lambda: cast_chunk(1, 0, "q"),
            17: lambda: cast_chunk(1, 0, "t"),
            18: lambda: (transpose(1, 0), cast_chunk(1, 1, "q")),
            19: lambda: cast_chunk(1, 1, "t"),
            21: lambda: (transpose(1, 1), cast_v(1, 0)),
            23: lambda: cast_v(1, 1),
        }

        state = {}

        def qk_group(b, qb, g):
            q0 = qb * QB
            qt_tp = qt_tps[b]
            s_ps = qk_psum.tile([128, 1024], F32, tag="qk")
            num = num_pool.tile([128, 1024], BF16, tag="num")
            act_spans = []      # merged contiguous spans (left-packed)
            mask_blocks = []    # span starts of diagonal blocks
            last_g = (g == (4 * qb + 4) // 2 - 1)
            for j, c in enumerate((2 * g, 2 * g + 1)):
                i = c - 4 * qb
                lo = 128 * i if i > 0 else 0
                w = QB - lo
                ql = q0 + lo
                t0_ = ql // 128
                nt = (QB - lo) // 128
                # the final (i2,i3) pair packs into one bank: j1 at col 256
                s0 = 256 if (last_g and j == 1) else j * 512
                rhs = qt_tp[:, q_slot(t0_):q_slot(t0_) + nt, :] \
                    .rearrange("p t q -> p (t q)")
                nc.tensor.matmul(
                    s_ps[:, s0:s0 + w],
                    lhsT=qt_tp[:, t_slot(c), :],
                    rhs=rhs,
                    start=not (last_g and j == 1), stop=True,
                    skip_group_check=(last_g and j == 1),
                )
                if act_spans and act_spans[-1][1] == s0:
                    act_spans[-1] = (act_spans[-1][0], s0 + w)
                else:
                    act_spans.append((s0, s0 + w))
                if i >= 0:
                    mask_blocks.append(s0)
            for lo_, hi_ in act_spans:
                nc.scalar.activation(num[:, lo_:hi_], s_ps[:, lo_:hi_],
                                     mybir.ActivationFunctionType.Exp,
                                     scale=SCALE)
                nc.vector.tensor_scalar_max(num[:, lo_:hi_],
                                            num[:, lo_:hi_], 1.0)
            for ds in mask_blocks:
                nc.vector.tensor_tensor(num[:, ds:ds + 128],
                                        num[:, ds:ds + 128], tri01[:],
                                        op=Alu.mult)
            st = state.setdefault((b, qb), {"ob": None, "num": {}})
            if st["ob"] is None:
                st["ob"] = [ob_psum.tile([128, 2, 256], F32, tag="ob",
                                         name=f"ob_{b}_{qb}_{h}")
                            for h in range(2)]
            st["num"][g] = num

        def pv_group(b, qb, g):
            st = state[(b, qb)]
            num = st["num"].pop(g)
            v_aug = v_augs[b]
            last_g = (g == (4 * qb + 4) // 2 - 1)
            for j, c in enumerate((2 * g, 2 * g + 1)):
                i = c - 4 * qb
                lo = 128 * i if i > 0 else 0
                s0 = 256 if (last_g and j == 1) else j * 512
                for sub in range(max(i, 0), 4):
                    ob = st["ob"][sub // 2]
                    nc.tensor.matmul(
                        ob[:, sub % 2, 0:129],
                        lhsT=num[:, s0 + sub * 128 - lo:
                                 s0 + (sub + 1) * 128 - lo],
                        rhs=v_aug[:, c, 0:129],
                        start=(c == 0 and sub % 2 == 0),
                        stop=(c == 4 * qb + sub),
                        skip_group_check=True,
                    )

        def finalize(b, qb):
            st = state.pop((b, qb))
            o_tile = fin_pool.tile([128, 4, 128], F32, tag="fin")
            for h in range(2):
                ob_sb = rec_pool.tile([128, 2, 129], F32, tag="rec")
                nc.vector.tensor_copy(ob_sb[:], st["ob"][h][:, :, 0:129])
                for s2 in range(2):
                    nc.gpsimd.normalize_recip(
                        o_tile[:, 2 * h + s2, :],
                        ob_sb[:, s2, 0:128],
                        ob_sb[:, s2, 128:129])
            nc.sync.dma_start(
                o_ext[b, qb * QB:(qb + 1) * QB, :]
                    .rearrange("(s p) d -> p s d", p=128),
                o_tile[:])

        pending = deque()

        def flush_one():
            b, qb, g = pending.popleft()
            pv_group(b, qb, g)
            if g == (4 * qb + 4) // 2 - 1:
                finalize(b, qb)

        n_items = len(items)
        for idx, it in enumerate(items):
            qk_group(*it)
            if idx in prep_at:
                prep_at[idx]()
            pending.append(it)
            # drain harder near the end so the tail is short
            depth = 2 if idx < n_items - 4 else 1
            while len(pending) > depth:
                flush_one()
        while pending:
            flush_one()

    nc.compile()
    return nc


_NC_CACHE = None


def _get_nc():
    global _NC_CACHE
    if _NC_CACHE is None:
        _NC_CACHE = build_attention_core()
    return _NC_CACHE


def kernel(Q: np.ndarray, T: np.ndarray, V: np.ndarray) -> np.ndarray:
    """Full-input entry point: shard over batch, run 8-core SPMD, gather."""
    from concourse.bass_utils import run_bass_kernel_spmd

    Q = np.ascontiguousarray(np.asarray(Q, dtype=np.float32))
    T = np.ascontiguousarray(np.asarray(T, dtype=np.float32))
    V = np.ascontiguousarray(np.asarray(V, dtype=np.float32))
    assert Q.shape == (B, S, D), Q.shape

    nc = _get_nc()
    in_maps = [
        {
            "Q": Q[i * B_LOC:(i + 1) * B_LOC],
            "T": T[i * B_LOC:(i + 1) * B_LOC],
            "V": V[i * B_LOC:(i + 1) * B_LOC],
        }
        for i in range(N_CORES)
    ]
    res = run_bass_kernel_spmd(nc, in_maps, core_ids=list(range(N_CORES)))
    return np.concatenate([res.results[i]["out"] for i in range(N_CORES)], axis=0)

